# revision 1
# baseline (speedup 1.0000x reference)
"""LocallyConnected2d (64,64,32,32) x (1,64,64,32,32,9) -> (64,64,32,32) on 8 trn2 cores.

Strategy
--------
Spatial sharding over output rows: core i computes output rows [4i, 4i+4).

Per output location (x, y) the op is an independent GEMM:
    out[:, :, x, y] = patches(x,y) @ W(x,y).T + bias(:, x, y)
with contraction over (c, k) = 64*9 = 576, M = 64 out-channels, N = 64 batch.

On device, per location we issue 6 accumulating matmuls into PSUM:
  - x band lives in SBUF as [128, 64*204]: partitions 0-63 hold channels c
    (copy A), partitions 64-127 hold the same data shifted by +1 element
    (copy B), so a single K=128 matmul contracts over (c, two adjacent kernel
    taps) at once:
      chunk q in {0,1,2}: taps k=3q (copy A) and k=3q+1 (copy B), K=128
      single s in {0,1,2}: tap k=3s+2, K=64 (loc A on partitions 0-63,
      loc B on partitions 64-127 -- weights packed accordingly)
  - weights are host-prepacked to the exact [K, M] SBUF layout, streamed in
    8 blocks of 8 location-pairs.
  - bias is folded in with one K=8 indicator matmul per PSUM bank
    (psum[p, j*64+b] += bias_col_j[p] * ind[j, col]).

Outputs accumulate in PSUM banks of [128, 512] = 8 location-pairs, are
copied to SBUF by the vector engine and DMAed out in device-friendly
layout; the host untangles the layout at the end.

Compute dtype fp16 (fp32 accumulate in PSUM): 1 cycle/row on the PE vs 4
for fp32, and half the HBM traffic. |inputs| ~ N(0,1) so fp16 range is safe.
"""

import numpy as np

N_B, C, H, W_W, O = 64, 64, 32, 32, 64
KH = KW = 3
NCORES = 8
RPC = H // NCORES            # 4 output rows per core
BAND = RPC + 2               # 6 padded input rows per core
WP = W_W + 2                 # 34 padded width
XFREE = BAND * WP * N_B      # 13056, layout (h, w, b) -- b innermost
XPAD = 64                    # pad so the +1-w (=+64 elem) copy-B read is in bounds
NPAIR_CORE = RPC * W_W // 2  # 64 location pairs per core
NTILE = 8                    # PSUM tiles per core (8 pairs each)
PAIR_COLS = 576              # weight cols per location pair
W_FREE = NPAIR_CORE * PAIR_COLS  # 36864

COMPUTE_NP = np.float16      # np.float16 | np.float32 | ml_dtypes.bfloat16

_CACHE = {}


def _mybir_dt(np_dt):
    import concourse.mybir as mybir
    import ml_dtypes

    if np_dt == np.float16:
        return mybir.dt.float16
    if np_dt == np.float32:
        return mybir.dt.float32
    if np_dt == ml_dtypes.bfloat16:
        return mybir.dt.bfloat16
    raise ValueError(np_dt)


def build_nc(compute_np=None):
    """Build the (single-program) Bass kernel; same NEFF runs on all 8 cores."""
    import concourse.bass as bass  # noqa: F401
    import concourse.mybir as mybir
    import concourse.tile as tile
    from concourse import bacc
    from contextlib import ExitStack

    cdt = _mybir_dt(compute_np or COMPUTE_NP)
    f32 = mybir.dt.float32

    nc = bacc.Bacc("TRN2", target_bir_lowering=False, debug=False)

    x_dram = nc.dram_tensor("xb", [64, XFREE + XPAD], cdt, kind="ExternalInput")
    w_dram = nc.dram_tensor("wp", [128, W_FREE], cdt, kind="ExternalInput")
    b_dram = nc.dram_tensor("bp", [8, NTILE * 128], cdt, kind="ExternalInput")
    i_dram = nc.dram_tensor("ind", [8, 512], cdt, kind="ExternalInput")
    o_dram = nc.dram_tensor("out", [NTILE, 128, 512], f32, kind="ExternalOutput")

    with ExitStack() as ctx:
        tc = ctx.enter_context(tile.TileContext(nc))
        const = ctx.enter_context(tc.tile_pool(name="const", bufs=1))
        wpool = ctx.enter_context(tc.tile_pool(name="wpool", bufs=4))
        ppool = ctx.enter_context(tc.tile_pool(name="ppool", bufs=6, space="PSUM"))
        spool = ctx.enter_context(tc.tile_pool(name="spool", bufs=4))

        # x free layout: f = (h*34 + w)*64 + b -- batch innermost so matmul
        # rhs columns are contiguous (strided rhs measured 3x slower on PE).
        XH = 3 * WP * 64  # 6528, half the band (h rows 0-2)

        xsb = const.tile([128, XFREE], cdt)
        bias_sb = const.tile([8, NTILE * 128], cdt)
        ind_sb = const.tile([8, 512], cdt)
        # copy A (+0, partitions 0-63) on SP, copy B (+1 element, partitions
        # 64-127) on ACT: concurrent queues hit disjoint port halves. h-major
        # layout, chunked at h=3 so tile 0's rows land first.
        nc.sync.dma_start(xsb[0:64, 0:XH], x_dram.ap()[:, 0:XH])
        # copy B ( +1 w = +64 elems) built on-chip by the idle vector engine;
        # chunk ranges cover exactly the w<=32 reads each half needs.
        nc.vector.tensor_copy(xsb[64:128, 0 : XH - 64], xsb[0:64, 64:XH])
        nc.gpsimd.dma_start(bias_sb[:], b_dram.ap()[:, :])
        nc.gpsimd.dma_start(ind_sb[:], i_dram.ap()[:, :])

        x4 = xsb[:].rearrange("p (h w b) -> p h w b", h=BAND, w=WP)  # [128,6,34,64]

        for t in range(NTILE):
            wt = wpool.tile([128, 8 * PAIR_COLS], cdt)
            wbase = t * 8 * PAIR_COLS
            if t == 0:
                # split w0 so pair-0 matmuls can start before all 8 pairs land
                half = 4 * PAIR_COLS
                nc.sync.dma_start(
                    wt[:, 0:half], w_dram.ap()[:, wbase : wbase + half]
                )
                nc.sync.dma_start(
                    wt[:, half : 8 * PAIR_COLS],
                    w_dram.ap()[:, wbase + half : wbase + 8 * PAIR_COLS],
                )
                # second h-half of copy A after w0; copy B again via DVE
                nc.sync.dma_start(
                    xsb[0:64, XH:XFREE], x_dram.ap()[:, XH:XFREE]
                )
                nc.vector.tensor_copy(
                    xsb[64:128, XH - 64 : XFREE - 64], xsb[0:64, XH:XFREE]
                )
            else:
                weng = nc.sync if t % 2 == 0 else nc.scalar
                weng.dma_start(
                    wt[:], w_dram.ap()[:, wbase : wbase + 8 * PAIR_COLS]
                )
            ps = ppool.tile([128, 512], f32)
            xrow = t // 2
            for jp in range(8):
                jr = (t % 2) * 8 + jp       # pair index within the x-row
                yA = 2 * jr                 # w-offset of loc A
                base = jp * PAIR_COLS
                oc = jp * 64
                # loc A paired taps (k=3q copy A, k=3q+1 copy B), K=128
                for q in range(3):
                    nc.tensor.matmul(
                        ps[0:64, oc : oc + 64],
                        wt[:, base + q * 64 : base + (q + 1) * 64],
                        x4[:, xrow + q, yA, :],
                        start=(jp == 0 and q == 0),
                        stop=False,
                        skip_group_check=True,
                    )
                # loc B paired taps, K=128
                for q in range(3):
                    nc.tensor.matmul(
                        ps[64:128, oc : oc + 64],
                        wt[:, base + 192 + q * 64 : base + 192 + (q + 1) * 64],
                        x4[:, xrow + q, yA + 1, :],
                        start=(jp == 0 and q == 0),
                        stop=False,
                        skip_group_check=True,
                    )
                # single taps k=3s+2 (A rows 0-63 via copy A, B rows 64-127
                # via copy B), K=64 row-tiled
                for s in range(3):
                    sb = base + 384 + s * 64
                    nc.tensor.matmul(
                        ps[0:64, oc : oc + 64],
                        wt[0:64, sb : sb + 64],
                        x4[0:64, xrow + s, yA + 2, :],
                        start=False,
                        stop=False,
                        skip_group_check=True,
                    )
                    nc.tensor.matmul(
                        ps[64:128, oc : oc + 64],
                        wt[64:128, sb : sb + 64],
                        x4[64:128, xrow + s, yA + 2, :],
                        start=False,
                        stop=False,
                        skip_group_check=True,
                    )
            # bias: psum[p, j*64+b] += bias[j, t*128+p] * ind[j, col]
            nc.tensor.matmul(
                ps[:, :],
                bias_sb[:, t * 128 : (t + 1) * 128],
                ind_sb[:, :],
                start=False,
                stop=True,
                skip_group_check=True,
            )
            stg = spool.tile([128, 512], f32)
            nc.vector.tensor_copy(stg[:], ps[:])
            nc.sync.dma_start(o_dram.ap()[t], stg[:])

    nc.compile()
    return nc


def pack_inputs(x, weight, bias, compute_np=None):
    """Full fp32 inputs -> list of 8 per-core input dicts (device layouts)."""
    cnp = compute_np or COMPUTE_NP
    x = np.asarray(x)
    w5 = np.asarray(weight)[0]        # (o, c, x, y, k)
    b3 = np.asarray(bias)[0]          # (o, x, y)

    xp = np.pad(x, ((0, 0), (0, 0), (1, 1), (1, 1)))  # (b, c, 34, 34)

    ind = np.zeros((8, 512), dtype=cnp)
    for j in range(8):
        ind[j, j * 64 : (j + 1) * 64] = 1.0

    in_maps = []
    for i in range(NCORES):
        band = xp[:, :, RPC * i : RPC * i + BAND, :]          # (b, c, 6, 34)
        xb = np.ascontiguousarray(band.transpose(1, 2, 3, 0)) # (c, 6, 34, b)
        xb = xb.astype(cnp).reshape(64, XFREE)
        # trailing pad so the +64-element copy-B read stays in bounds
        xb = np.concatenate([xb, np.zeros((64, XPAD), dtype=cnp)], axis=1)

        wc = w5[:, :, RPC * i : RPC * (i + 1), :, :]          # (o, c, 4, 32, 9)
        wcr = wc.reshape(64, 64, 4, 16, 2, 9)                 # o c xh jr ab k
        chunks = wcr[..., [0, 1, 3, 4, 6, 7]].reshape(64, 64, 4, 16, 2, 3, 2)
        # -> [p=(half,c), j=(xh,jr), col=(ab,q,o)]
        chunks = chunks.transpose(6, 1, 2, 3, 4, 5, 0).reshape(128, 64, 384)
        singles = wcr[..., [2, 5, 8]]                         # o c xh jr ab s
        # -> [p=(ab,c), j=(xh,jr), col=(s,o)]
        singles = singles.transpose(4, 1, 2, 3, 5, 0).reshape(128, 64, 192)
        wp = np.concatenate([chunks, singles], axis=2)        # (128, 64, 576)
        wp = np.ascontiguousarray(wp).astype(cnp).reshape(128, W_FREE)

        bc = b3[:, RPC * i : RPC * (i + 1), :]                # (o, 4, 32)
        bcr = bc.reshape(64, 4, 2, 8, 2)                      # o xh half j' hi
        bp = bcr.transpose(3, 1, 2, 4, 0).reshape(8, NTILE * 128).astype(cnp)

        in_maps.append(
            {
                "xb": xb,
                "wp": wp,
                "bp": np.ascontiguousarray(bp),
                "ind": ind,
            }
        )
    return in_maps


def unpack_output(core_outs):
    """8 per-core [NTILE,128,512] arrays -> full (64, 64, 32, 32) output."""
    arr = np.stack(core_outs)                     # (core, t, p, col)
    arr = arr.reshape(8, 4, 2, 2, 64, 8, 64)      # core xh half hi o j' b
    out = arr.transpose(6, 4, 0, 1, 2, 5, 3)      # b o core xh half j' hi
    return np.ascontiguousarray(out.reshape(64, 64, 32, 32), dtype=np.float32)


def run_on_device(in_maps, trace=False, compute_np=None, **kwargs):
    from concourse import bass_utils

    key = ("nc", np.dtype(compute_np or COMPUTE_NP).name)
    if key not in _CACHE:
        _CACHE[key] = build_nc(compute_np)
    nc = _CACHE[key]
    res = bass_utils.run_bass_kernel_spmd(
        nc, in_maps, core_ids=list(range(NCORES)), trace=trace, **kwargs
    )
    return res


def kernel(x, weight, bias):
    in_maps = pack_inputs(x, weight, bias)
    res = run_on_device(in_maps)
    return unpack_output([r["out"] for r in res.results])



# revision 6
# speedup vs baseline: 1.0157x; 1.0157x over previous
"""LocallyConnected2d (64,64,32,32) x (1,64,64,32,32,9) -> (64,64,32,32) on 8 trn2 cores.

Strategy (v2: x-stationary)
---------------------------
Spatial sharding over output rows: core i computes output rows [4i, 4i+4).

The op is per-location GEMMs out[b, o] += sum_{c,kh,kw} x(c, X+kh, Y+kw) W(o, c, kh, kw).
v1 kept W stationary in the PE array: LDWEIGHTS fill time scales with the
number of stationary COLUMNS loaded (P/1.2GHz), and every W element had to be
filled once -> fill-bound at ~41us/core.

v2 makes X the stationary operand and streams W as the moving tensor:
  stationary S(r, j) = [K=128, M=64]: partitions 0-63 = x(c, row r, col j),
  partitions 64-127 = x(c, row r+1, col j) (row-shifted copy B); M = batch.
  moving = W col-blocks [K, N]: N = 64*len(locs) output-channel blocks.
  out[m=batch, n=(loc, oc)] accumulates in PSUM: one [64, 64] block per
  location; all 8 psum banks hold the core's full 4x32 locations.

Tap coverage per loc (X, Y):
  - "dominoes" (kh=0,1) x kw=0..2: full-K matmuls at sweep r=X, blocks j=Y+kw.
  - "singles" (kh=2) x kw: K=64 matmuls: X=0 upper / X=1 lower at sweep r=2,
    X=2 upper / X=3 lower at sweep r=4 (upper = copy A row r, lower = copy B
    row r+1); upper and lower singles share W columns -> zero DMA waste.
Bias is folded in by initializing each psum bank with a K=1 matmul
(ones stationary, bias row moving) before its first accumulation.

Tensor cost ~24us/core sits under the DMA floor (~36us: W 9.4MB + x 1.7MB +
out 1MB fp16 at ~333 GB/s), so the kernel is scheduled for DMA saturation:
all W chunk DMAs are issued up front on two queues (every chunk has its own
SBUF buffer -- no ring reuse), x lands first on a third queue, outputs
stream out mid-kernel as their banks complete.

Compute dtype fp16 (fp32 accumulate in PSUM), output fp16.
"""

import numpy as np

N_B, C, H, W_W, O = 64, 64, 32, 32, 64
NCORES = 8
RPC = H // NCORES            # 4 output rows per core
BAND = RPC + 2               # 6 padded input rows per core
WP = W_W + 2                 # 34 padded width
ROWF = WP * N_B              # 2176 elems per band row (w, b), b innermost
XFREE = BAND * ROWF          # 13056
WCOLS = 36864                # total W stream columns per core
CHUNK_TARGET = 2048          # ~4KB/partition per W chunk DMA

COMPUTE_NP = np.float16

_CACHE = {}


def _locs(j):
    return [y for y in (j - 2, j - 1, j) if 0 <= y < W_W]


def _runs(locs):
    """Group consecutive locs by psum (bank,half) group Y//8."""
    runs = []
    for y in locs:
        if runs and runs[-1][-1] == y - 1 and (runs[-1][0] // 8) == (y // 8):
            runs[-1].append(y)
        else:
            runs.append([y])
    return runs


def plan_core():
    """Shared layout plan: ordered W column stream + MM descriptors.

    Returns (wblocks, mms, chunks):
      wblocks: ordered list of (r, j, kind, Y) with kind in {dom, sng};
               each owns 64 W columns (dom: both halves kh0/kh1; sng: upper
               half = X_up single, lower half = X_lo single).
      mms: ordered MM descriptors dicts.
      chunks: list of (col_start, col_end) chunk boundaries.
    """
    wblocks = []
    mms = []
    col = 0
    chunk_bounds = []
    chunk_start = 0

    def close_chunk():
        nonlocal chunk_start
        if col > chunk_start:
            chunk_bounds.append((chunk_start, col))
            chunk_start = col

    for r in range(5):
        for j in range(WP):
            locs = _locs(j)
            unit_mms = []
            if r <= 3:  # dominoes for X=r
                X = r
                base = col
                for y in locs:
                    wblocks.append((r, j, "dom", y))
                col += 64 * len(locs)
                off = 0
                for run in _runs(locs):
                    y0 = run[0]
                    unit_mms.append(dict(
                        kind="dom", r=r, j=j, X=X,
                        wc0=base + off, n=64 * len(run),
                        plo=0, phi=128,
                        bank=2 * X + y0 // 16,
                        poff=64 * ((y0 // 8) % 2),
                        pc0=(y0 % 8) * 64,
                        ys=list(run),
                    ))
                    off += 64 * len(run)
            if r in (2, 4):  # singles: upper X_up, lower X_lo share cols
                XU, XL = (0, 1) if r == 2 else (2, 3)
                base = col
                for y in locs:
                    wblocks.append((r, j, "sng", y))
                col += 64 * len(locs)
                for half, X, plo in ((0, XU, 0), (1, XL, 64)):
                    off = 0
                    for run in _runs(locs):
                        y0 = run[0]
                        unit_mms.append(dict(
                            kind="sng", r=r, j=j, X=X,
                            wc0=base + off, n=64 * len(run),
                            plo=plo, phi=plo + 64,
                            bank=2 * X + y0 // 16,
                            poff=64 * ((y0 // 8) % 2),
                            pc0=(y0 % 8) * 64,
                            ys=list(run),
                        ))
                        off += 64 * len(run)
            mms.extend(unit_mms)
            if col - chunk_start >= CHUNK_TARGET:
                close_chunk()
    close_chunk()
    assert col == WCOLS, col
    return wblocks, mms, chunk_bounds


def _mybir_dt(np_dt):
    import concourse.mybir as mybir
    import ml_dtypes

    if np_dt == np.float16:
        return mybir.dt.float16
    if np_dt == np.float32:
        return mybir.dt.float32
    if np_dt == ml_dtypes.bfloat16:
        return mybir.dt.bfloat16
    raise ValueError(np_dt)


def build_nc(compute_np=None):
    """Build the (single-program) Bass kernel; same NEFF runs on all 8 cores."""
    import concourse.bass as bass  # noqa: F401
    import concourse.mybir as mybir
    import concourse.tile as tile
    from concourse import bacc
    from contextlib import ExitStack

    cdt = _mybir_dt(compute_np or COMPUTE_NP)
    f32 = mybir.dt.float32

    _, mms, chunks = plan_core()
    nchunks = len(chunks)

    nc = bacc.Bacc("TRN2", target_bir_lowering=False, debug=False)

    x_dram = nc.dram_tensor("xb", [64, XFREE], cdt, kind="ExternalInput")
    w_dram = nc.dram_tensor("wp", [128, WCOLS], cdt, kind="ExternalInput")
    b_dram = nc.dram_tensor("bp", [1, 16 * 512], cdt, kind="ExternalInput")
    o_dram = nc.dram_tensor("out", [8, 128, 512], cdt, kind="ExternalOutput")

    with ExitStack() as ctx:
        tc = ctx.enter_context(tile.TileContext(nc))
        const = ctx.enter_context(tc.tile_pool(name="const", bufs=1))
        wpool = ctx.enter_context(tc.tile_pool(name="wpool", bufs=1))
        ppool = ctx.enter_context(tc.tile_pool(name="ppool", bufs=1, space="PSUM"))
        spool = ctx.enter_context(tc.tile_pool(name="spool", bufs=1))

        xsb = const.tile([128, XFREE], cdt)
        bias_sb = const.tile([1, 16 * 512], cdt)
        ones_sb = const.tile([1, 64], cdt)

        nc.gpsimd.memset(ones_sb[:], 1.0)

        # x copy A (partitions 0-63): rows 0-2 first so sweeps 0-1 unblock,
        # then rows 3-5. bias rides ahead of x on the same queue (tiny).
        nc.sync.dma_start(bias_sb[:], b_dram.ap()[:, :])
        nc.sync.dma_start(xsb[0:64, 0 : 3 * ROWF], x_dram.ap()[:, 0 : 3 * ROWF])
        nc.sync.dma_start(
            xsb[0:64, 3 * ROWF : XFREE], x_dram.ap()[:, 3 * ROWF : XFREE]
        )
        # copy B = row-shifted copy A on partitions 64-127 (idle vector engine)
        nc.vector.tensor_copy(
            xsb[64:128, 0 : 2 * ROWF], xsb[0:64, ROWF : 3 * ROWF]
        )
        nc.vector.tensor_copy(
            xsb[64:128, 2 * ROWF : 5 * ROWF], xsb[0:64, 3 * ROWF : XFREE]
        )

        # all W chunk DMAs issued up front, alternating two queues
        wtiles = []
        for ci, (c0, c1) in enumerate(chunks):
            wt = wpool.tile([128, c1 - c0], cdt, name=f"wt{ci}")
            eng = nc.scalar if ci % 2 == 0 else nc.gpsimd
            eng.dma_start(wt[:], w_dram.ap()[:, c0:c1])
            wtiles.append(wt)

        ps = [ppool.tile([128, 512], f32, name=f"ps{b}") for b in range(8)]

        def bias_init(bank):
            for half in range(2):
                nc.tensor.matmul(
                    ps[bank][64 * half : 64 * half + 64, 0:512],
                    ones_sb[0:1, 0:64],
                    bias_sb[0:1, (2 * bank + half) * 512 : (2 * bank + half + 1) * 512],
                    start=True,
                    stop=False,
                    skip_group_check=True,
                )

        # chunk lookup for a given W col
        def chunk_of(c):
            for ci, (c0, c1) in enumerate(chunks):
                if c0 <= c < c1:
                    return ci, c - c0
            raise ValueError(c)

        x4 = xsb[:].rearrange("p (h w b) -> p h w b", h=BAND, w=WP)

        emitted_init = set()

        def ensure_init(r):
            # bank pair 2X,2X+1 must be bias-initialized before sweep r=X
            # dominoes (and 6,7 before sweep 3)
            if r <= 3:
                for bank in (2 * r, 2 * r + 1):
                    if bank not in emitted_init:
                        bias_init(bank)
                        emitted_init.add(bank)

        cur_r = -1
        out_done = 0

        def flush_outputs(upto):
            # copy finished banks to SBUF fp16 and DMA out
            nonlocal out_done
            for bank in range(out_done, upto):
                stg = spool.tile([128, 512], cdt, name=f"stg{bank}")
                nc.vector.tensor_copy(stg[:], ps[bank][:])
                nc.sync.dma_start(o_dram.ap()[bank], stg[:])
            out_done = upto

        for m in mms:
            if m["r"] != cur_r:
                cur_r = m["r"]
                ensure_init(cur_r)
                if cur_r == 3:
                    # banks 0-3 (X=0,1) final after sweep 2
                    flush_outputs(4)
            ci, loc0 = chunk_of(m["wc0"])
            assert m["wc0"] + m["n"] <= chunks[ci][1]
            wt = wtiles[ci]
            plo, phi = m["plo"], m["phi"]
            stat = x4[plo:phi, m["r"], m["j"], :]
            mov = wt[plo:phi, loc0 : loc0 + m["n"]]
            out = ps[m["bank"]][
                m["poff"] : m["poff"] + 64, m["pc0"] : m["pc0"] + m["n"]
            ]
            nc.tensor.matmul(
                out, stat, mov, start=False, stop=False, skip_group_check=True
            )
        flush_outputs(8)

    nc.compile()
    return nc


def pack_inputs(x, weight, bias, compute_np=None):
    """Full fp32 inputs -> list of 8 per-core input dicts (device layouts)."""
    cnp = compute_np or COMPUTE_NP
    x = np.asarray(x)
    w5 = np.asarray(weight)[0]        # (o, c, X, Y, k)
    b3 = np.asarray(bias)[0]          # (o, X, Y)

    xp = np.pad(x, ((0, 0), (0, 0), (1, 1), (1, 1)))  # (b, c, 34, 34)
    wblocks, _, _ = plan_core()

    # w5 transposed to (X, Y, k, c, o) so each 64-col block is a contiguous-ish
    # [c, o] slice
    w5t = np.ascontiguousarray(w5.transpose(2, 3, 4, 1, 0)).astype(cnp)

    in_maps = []
    for i in range(NCORES):
        band = xp[:, :, RPC * i : RPC * i + BAND, :]          # (b, c, 6, 34)
        xb = np.ascontiguousarray(band.transpose(1, 2, 3, 0)) # (c, 6, 34, b)
        xb = xb.astype(cnp).reshape(64, XFREE)

        wp = np.empty((128, WCOLS), dtype=cnp)
        col = 0
        for (r, j, kind, y) in wblocks:
            if kind == "dom":
                X = 4 * i + r
                wp[0:64, col : col + 64] = w5t[X, y, (j - y)]
                wp[64:128, col : col + 64] = w5t[X, y, 3 + (j - y)]
            else:
                XU, XL = (0, 1) if r == 2 else (2, 3)
                wp[0:64, col : col + 64] = w5t[4 * i + XU, y, 6 + (j - y)]
                wp[64:128, col : col + 64] = w5t[4 * i + XL, y, 6 + (j - y)]
            col += 64
        assert col == WCOLS

        # bias: [1, 16*512]: (bank, half) -> 8 locs x 64 oc
        bp = np.empty((1, 16 * 512), dtype=cnp)
        for bank in range(8):
            X = bank // 2
            for half in range(2):
                pb = (bank % 2) * 2 + half
                ys = slice(pb * 8, pb * 8 + 8)
                blk = b3[:, 4 * i + X, ys]                    # (o, 8)
                bp[0, (2 * bank + half) * 512 : (2 * bank + half + 1) * 512] = (
                    blk.T.reshape(-1).astype(cnp)
                )

        in_maps.append({"xb": xb, "wp": wp, "bp": bp})
    return in_maps


def unpack_output(core_outs):
    """8 per-core [8, 128, 512] arrays -> full (64, 64, 32, 32) output."""
    arr = np.stack(core_outs)                      # (core, bank, p, col)
    # bank = 2X + hb ; p = 64*half + b ; col = y8*64 + o
    arr = arr.reshape(8, 4, 2, 2, 64, 8, 64)       # core X hb half b y8 o
    # Y = hb*16 + half*8 + y8 ; Hrow = core*4 + X
    out = arr.transpose(4, 6, 0, 1, 2, 3, 5)       # b o core X hb half y8
    return np.ascontiguousarray(
        out.reshape(64, 64, 32, 32), dtype=np.float32
    )


def run_on_device(in_maps, trace=False, compute_np=None, **kwargs):
    from concourse import bass_utils

    key = ("nc", np.dtype(compute_np or COMPUTE_NP).name)
    if key not in _CACHE:
        _CACHE[key] = build_nc(compute_np)
    nc = _CACHE[key]
    res = bass_utils.run_bass_kernel_spmd(
        nc, in_maps, core_ids=list(range(NCORES)), trace=trace, **kwargs
    )
    return res


def kernel(x, weight, bias):
    in_maps = pack_inputs(x, weight, bias)
    res = run_on_device(in_maps)
    return unpack_output([r["out"] for r in res.results])


# revision 7
# speedup vs baseline: 1.1862x; 1.1679x over previous
"""LocallyConnected2d (64,64,32,32) x (1,64,64,32,32,9) -> (64,64,32,32) on 8 trn2 cores.

Strategy (v3: x-stationary, parity-alternating PE column groups)
----------------------------------------------------------------
Spatial sharding over output rows: core i computes output rows [4i, 4i+4).

The op is per-location GEMMs out[b, o] += sum_{c,kh,kw} x(c, X+kh, Y+kw) W(o, c, kh, kw).
W-stationary is LDWEIGHTS-fill bound (fill ~ cols/1.2GHz, every W element
loaded once -> ~41us/core). So X is the stationary operand and W streams:

  stationary S(r, j) = [K=128, M=64]: partitions 0-63 = x(c, row r, col j),
  partitions 64-127 = x(c, row r+1, col j) (row-shifted copy B); M = batch.
  moving = W col-blocks [K, N]; out[m=batch, n=(loc, oc)] accumulates in PSUM.

Tap coverage per loc (X, Y):
  - dominoes (kh=0,1) x kw: full-K matmuls at sweep r=X, blocks j=Y+kw.
  - singles (kh=2) x kw: K=64 matmuls: X=0 upper/X=1 lower at sweep r=2,
    X=2 upper/X=3 lower at sweep r=4; upper and lower share W columns.

PSUM layout puts loc parity (Y%2) on the partition half, so per block the
two matmuls (even locs -> partitions 0-63, odd -> 64-127) target opposite
PE column groups: each matmul's stationary fill overlaps the previous
matmul's stream (the PE only overlaps LDWEIGHTS with in-flight MATMULs on
non-conflicting array tiles). Without this, every fill serializes (~53ns x
~550 fills).

Each sweep runs as passes (dominoes, singles-upper, singles-lower) with W
laid out in exact consumption order; banks flush (fp32->fp16 cast + DMA
out) as soon as their output row completes, spreading the output DMA.
Bias is folded in by initializing each psum bank with a K=1 matmul.

DMA floor ~36.5us/core (W 9.4MB + x 1.7MB + out 1MB fp16 at ~330GB/s);
tensor ~32us hides under it. All W chunk DMAs are issued up front on two
queues into dedicated SBUF buffers (no ring reuse), x lands first.
"""

import numpy as np

N_B, C, H, W_W, O = 64, 64, 32, 32, 64
NCORES = 8
RPC = H // NCORES            # 4 output rows per core
BAND = RPC + 2               # 6 padded input rows per core
WP = W_W + 2                 # 34 padded width
ROWF = WP * N_B              # 2176 elems per band row (w, b), b innermost
XFREE = BAND * ROWF          # 13056
WCOLS = 36864                # total W stream columns per core
CHUNK_TARGET = 2048          # ~4KB/partition per W chunk DMA

COMPUTE_NP = np.float16

_CACHE = {}


def _locs(j):
    return [y for y in (j - 2, j - 1, j) if 0 <= y < W_W]


def _pgroups(j):
    """Per block: even-parity locs first, then odd (fixed col-group order),
    each split at psum bank boundaries (Y//16). Returns list of loc-runs."""
    out = []
    for p in (0, 1):
        run = []
        for y in _locs(j):
            if y % 2 != p:
                continue
            if run and (run[-1] // 16) == (y // 16):
                run.append(y)
            else:
                if run:
                    out.append(run)
                run = [y]
        if run:
            out.append(run)
    return out


def _mm_rec(kind, r, j, run, X, plo, wc0):
    y0 = run[0]
    return dict(
        kind=kind, r=r, j=j,
        plo=plo, phi=plo + (128 if kind == "dom" else 64),
        wc0=wc0, n=64 * len(run),
        bank=2 * X + y0 // 16,
        poff=64 * (y0 % 2),
        pc0=((y0 % 16) // 2) * 64,
        ys=list(run),
    )


def plan_core():
    """Layout plan shared by builder and host packer.

    Returns (wblocks, prog, chunks):
      wblocks: ordered 64-col W blocks: (r, j, kind, Y); dom blocks carry
               (kh0 upper, kh1 lower) of X=r; sng blocks carry kh2 of X_up
               on the upper half and kh2 of X_lo on the lower half.
      prog: ordered program entries: MM dicts and flush markers.
      chunks: (col_start, col_end) W chunk DMA boundaries.
    """
    wblocks = []
    prog = []
    col = 0
    chunks = []
    chunk_start = 0

    def close_chunk():
        nonlocal chunk_start
        if col > chunk_start:
            chunks.append((chunk_start, col))
            chunk_start = col

    def maybe_chunk():
        if col - chunk_start >= CHUNK_TARGET:
            close_chunk()

    for r in range(5):
        if r <= 3:  # pass A: dominoes for X=r
            X = r
            for j in range(WP):
                for run in _pgroups(j):
                    for y in run:
                        wblocks.append((r, j, "dom", y))
                    prog.append(_mm_rec("dom", r, j, run, X, 0, col))
                    col += 64 * len(run)
                maybe_chunk()
        if r in (2, 4):  # passes B/C: singles, shared W cols
            XU, XL = (0, 1) if r == 2 else (2, 3)
            passB, passC = [], []
            for j in range(WP):
                for run in _pgroups(j):
                    for y in run:
                        wblocks.append((r, j, "sng", y))
                    passB.append(_mm_rec("sng", r, j, run, XU, 0, col))
                    passC.append(_mm_rec("sng", r, j, run, XL, 64, col))
                    col += 64 * len(run)
                maybe_chunk()
            prog.extend(passB)
            prog.append(dict(kind="flush", banks=[2 * XU, 2 * XU + 1]))
            prog.extend(passC)
            prog.append(dict(kind="flush", banks=[2 * XL, 2 * XL + 1]))
    close_chunk()
    assert col == WCOLS, col
    return wblocks, prog, chunks


def _mybir_dt(np_dt):
    import concourse.mybir as mybir
    import ml_dtypes

    if np_dt == np.float16:
        return mybir.dt.float16
    if np_dt == np.float32:
        return mybir.dt.float32
    if np_dt == ml_dtypes.bfloat16:
        return mybir.dt.bfloat16
    raise ValueError(np_dt)


def build_nc(compute_np=None):
    """Build the (single-program) Bass kernel; same NEFF runs on all 8 cores."""
    import concourse.bass as bass  # noqa: F401
    import concourse.mybir as mybir
    import concourse.tile as tile
    from concourse import bacc
    from contextlib import ExitStack

    cdt = _mybir_dt(compute_np or COMPUTE_NP)
    f32 = mybir.dt.float32

    _, prog, chunks = plan_core()

    nc = bacc.Bacc("TRN2", target_bir_lowering=False, debug=False)

    x_dram = nc.dram_tensor("xb", [64, XFREE], cdt, kind="ExternalInput")
    w_dram = nc.dram_tensor("wp", [128, WCOLS], cdt, kind="ExternalInput")
    b_dram = nc.dram_tensor("bp", [1, 16 * 512], cdt, kind="ExternalInput")
    o_dram = nc.dram_tensor("out", [8, 128, 512], cdt, kind="ExternalOutput")

    with ExitStack() as ctx:
        tc = ctx.enter_context(tile.TileContext(nc))
        const = ctx.enter_context(tc.tile_pool(name="const", bufs=1))
        wpool = ctx.enter_context(tc.tile_pool(name="wpool", bufs=1))
        ppool = ctx.enter_context(tc.tile_pool(name="ppool", bufs=1, space="PSUM"))
        spool = ctx.enter_context(tc.tile_pool(name="spool", bufs=1))

        xsb = const.tile([128, XFREE], cdt)
        bias_sb = const.tile([1, 16 * 512], cdt)
        ones_sb = const.tile([1, 64], cdt)

        nc.gpsimd.memset(ones_sb[:], 1.0)

        # x copy A (partitions 0-63): rows 0-2 first so sweeps 0-1 unblock.
        nc.sync.dma_start(bias_sb[:], b_dram.ap()[:, :])
        nc.sync.dma_start(xsb[0:64, 0 : 3 * ROWF], x_dram.ap()[:, 0 : 3 * ROWF])
        nc.sync.dma_start(
            xsb[0:64, 3 * ROWF : XFREE], x_dram.ap()[:, 3 * ROWF : XFREE]
        )
        # copy B = row-shifted copy A on partitions 64-127 (idle vector engine)
        nc.vector.tensor_copy(
            xsb[64:128, 0 : 2 * ROWF], xsb[0:64, ROWF : 3 * ROWF]
        )
        nc.vector.tensor_copy(
            xsb[64:128, 2 * ROWF : 5 * ROWF], xsb[0:64, 3 * ROWF : XFREE]
        )

        # all W chunk DMAs issued up front, alternating two queues
        wtiles = []
        for ci, (c0, c1) in enumerate(chunks):
            wt = wpool.tile([128, c1 - c0], cdt, name=f"wt{ci}")
            eng = nc.scalar if ci % 2 == 0 else nc.gpsimd
            eng.dma_start(wt[:], w_dram.ap()[:, c0:c1])
            wtiles.append(wt)

        ps = [ppool.tile([128, 512], f32, name=f"ps{b}") for b in range(8)]

        def bias_init(bank):
            # psum[p, n] = bias[n] for all partitions p (K=1 ones stationary);
            # alternate halves so fills alternate PE col groups too
            for half in range(2):
                nc.tensor.matmul(
                    ps[bank][64 * half : 64 * half + 64, 0:512],
                    ones_sb[0:1, 0:64],
                    bias_sb[0:1, (2 * bank + half) * 512 : (2 * bank + half + 1) * 512],
                    start=True,
                    stop=False,
                    skip_group_check=True,
                )

        def chunk_of(c):
            for ci, (c0, c1) in enumerate(chunks):
                if c0 <= c < c1:
                    return ci, c - c0
            raise ValueError(c)

        x4 = xsb[:].rearrange("p (h w b) -> p h w b", h=BAND, w=WP)

        emitted_init = set()

        def ensure_init(r):
            if r <= 3:
                for bank in (2 * r, 2 * r + 1):
                    if bank not in emitted_init:
                        bias_init(bank)
                        emitted_init.add(bank)

        flushed = set()

        def flush(banks):
            for bank in banks:
                if bank in flushed:
                    continue
                flushed.add(bank)
                stg = spool.tile([128, 512], cdt, name=f"stg{bank}")
                nc.vector.tensor_copy(stg[:], ps[bank][:])
                nc.sync.dma_start(o_dram.ap()[bank], stg[:])

        cur_r = -1
        for m in prog:
            if m["kind"] == "flush":
                flush(m["banks"])
                continue
            if m["r"] != cur_r:
                cur_r = m["r"]
                ensure_init(cur_r)
            ci, loc0 = chunk_of(m["wc0"])
            assert m["wc0"] + m["n"] <= chunks[ci][1]
            wt = wtiles[ci]
            plo, phi = m["plo"], m["phi"]
            stat = x4[plo:phi, m["r"], m["j"], :]
            mov = wt[plo:phi, loc0 : loc0 + m["n"]]
            out = ps[m["bank"]][
                m["poff"] : m["poff"] + 64, m["pc0"] : m["pc0"] + m["n"]
            ]
            nc.tensor.matmul(
                out, stat, mov, start=False, stop=False, skip_group_check=True
            )
        flush(range(8))

    nc.compile()
    return nc


def pack_inputs(x, weight, bias, compute_np=None):
    """Full fp32 inputs -> list of 8 per-core input dicts (device layouts)."""
    cnp = compute_np or COMPUTE_NP
    x = np.asarray(x)
    w5 = np.asarray(weight)[0]        # (o, c, X, Y, k)
    b3 = np.asarray(bias)[0]          # (o, X, Y)

    xp = np.pad(x, ((0, 0), (0, 0), (1, 1), (1, 1)))  # (b, c, 34, 34)
    wblocks, _, _ = plan_core()

    # (X, Y, k, c, o): each 64-col W block is w5t[X, y, k] = [c, o]
    w5t = np.ascontiguousarray(w5.transpose(2, 3, 4, 1, 0)).astype(cnp)

    in_maps = []
    for i in range(NCORES):
        band = xp[:, :, RPC * i : RPC * i + BAND, :]          # (b, c, 6, 34)
        xb = np.ascontiguousarray(band.transpose(1, 2, 3, 0)) # (c, 6, 34, b)
        xb = xb.astype(cnp).reshape(64, XFREE)

        wp = np.empty((128, WCOLS), dtype=cnp)
        col = 0
        for (r, j, kind, y) in wblocks:
            if kind == "dom":
                X = 4 * i + r
                wp[0:64, col : col + 64] = w5t[X, y, (j - y)]
                wp[64:128, col : col + 64] = w5t[X, y, 3 + (j - y)]
            else:
                XU, XL = (0, 1) if r == 2 else (2, 3)
                wp[0:64, col : col + 64] = w5t[4 * i + XU, y, 6 + (j - y)]
                wp[64:128, col : col + 64] = w5t[4 * i + XL, y, 6 + (j - y)]
            col += 64
        assert col == WCOLS

        # bias: [1, 16*512]: (bank, half) -> 8 locs x 64 oc
        # bank = 2X + Y//16 ; half = Y%2 ; col = ((Y%16)//2)*64 + oc
        bp = np.empty((1, 16 * 512), dtype=cnp)
        for bank in range(8):
            X = bank // 2
            for half in range(2):
                ys = np.arange(16 * (bank % 2) + half, 16 * (bank % 2) + 16, 2)
                blk = b3[:, 4 * i + X, ys]                    # (o, 8)
                bp[0, (2 * bank + half) * 512 : (2 * bank + half + 1) * 512] = (
                    blk.T.reshape(-1).astype(cnp)
                )

        in_maps.append({"xb": xb, "wp": wp, "bp": bp})
    return in_maps


def unpack_output(core_outs):
    """8 per-core [8, 128, 512] arrays -> full (64, 64, 32, 32) output."""
    arr = np.stack(core_outs)                      # (core, bank, p, col)
    # bank = 2X + hb ; p = 64*(Y%2) + b ; col = ((Y%16)//2)*64 + o
    arr = arr.reshape(8, 4, 2, 2, 64, 8, 64)       # core X hb par b q o
    # Y = hb*16 + q*2 + par
    out = arr.transpose(4, 6, 0, 1, 2, 5, 3)       # b o core X hb q par
    return np.ascontiguousarray(
        out.reshape(64, 64, 32, 32), dtype=np.float32
    )


def run_on_device(in_maps, trace=False, compute_np=None, **kwargs):
    from concourse import bass_utils

    key = ("nc", np.dtype(compute_np or COMPUTE_NP).name)
    if key not in _CACHE:
        _CACHE[key] = build_nc(compute_np)
    nc = _CACHE[key]
    res = bass_utils.run_bass_kernel_spmd(
        nc, in_maps, core_ids=list(range(NCORES)), trace=trace, **kwargs
    )
    return res


def kernel(x, weight, bias):
    in_maps = pack_inputs(x, weight, bias)
    res = run_on_device(in_maps)
    return unpack_output([r["out"] for r in res.results])
